# revision 1
# baseline (speedup 1.0000x reference)
"""Membership-norm kernel for Trainium2 (8 NeuronCores, data-parallel over N).

Computes out[n, c, w] = max(exp(-sum_d lamda[d,c] * (x[n,d,w] - c[d,c])^2), 1e-6)
for x: (8, 64, 16384) f32, c/lamda: (64, 80) f32 -> out: (8, 80, 16384) f32.

Sharding: core n processes batch element n (x[n]: (64, 16384) -> out[n]: (80, 16384)).

Fully bf16 I/O: x cast to bf16 on the HOST (2 MiB/core loads), output stored
bf16 (2.5 MiB/core) and upcast on the host; bf16's 2^-8 relative error is far
inside the 2e-2 gate. Loads alternate the SBUF partition half holding x;
per 2048-col group: DVE squares into the other half (2x bf16), one K=128 bf16
matmul per 512 cols with stationary [lamda; -2*lamda*c] (column-swapped
variant for lo tiles), ACT exp (scale=-1, bias=-sum lamda*c^2) writing bf16,
DVE max(.,1e-6) in 4x mode, SWDGE store (dispatch on the idle Pool queue).
Emission is software-pipelined one group ahead so the in-order DVE queue
never blocks squares of group g+1 behind the clip of group g.
"""

import sys

if "/opt/trn_rl_repo" not in sys.path:
    sys.path.insert(0, "/opt/trn_rl_repo")

import numpy as np

N, D, WH, C = 8, 64, 16384, 80
GROUP = 2048
MM_F = 512
LOAD_COLS = 4096
NG = WH // GROUP
NL = WH // LOAD_COLS
LOAD_HALF = ["lo", "hi", "hi", "hi"]

_cache = {}


def _build():
    import concourse.bass as bass
    import concourse.tile as tile
    from concourse import bacc, mybir

    f32 = mybir.dt.float32
    bf16 = mybir.dt.bfloat16
    Exp = mybir.ActivationFunctionType.Exp

    nc = bacc.Bacc("TRN2", target_bir_lowering=False, debug=False,
                   enable_asserts=False, enable_partition_id=False)

    xs_d = nc.dram_tensor("xs", [D, WH], bf16, kind="ExternalInput").ap()
    w_d = nc.dram_tensor("w", [2 * D, 2 * C], bf16, kind="ExternalInput").ap()
    nb_d = nc.dram_tensor("nb", [C, 1], f32, kind="ExternalInput").ap()
    out_d = nc.dram_tensor("out", [C, WH], bf16, kind="ExternalOutput").ap()

    with tile.TileContext(nc) as tc:
        with (
            tc.tile_pool(name="consts", bufs=1) as consts,
            tc.tile_pool(name="xp", bufs=NL) as xp,
            tc.tile_pool(name="op", bufs=4) as op,
            tc.tile_pool(name="pp", bufs=2, space="PSUM") as pp,
        ):
            ws = consts.tile([128, 2 * C], bf16)
            nbs = consts.tile([128, 1], f32)
            dummy = consts.tile([128, MM_F], bf16, name="dummy")
            scratch = consts.tile([8, 8], bf16, name="scratch")

            nc.sync.dma_start(ws[:, :], w_d[:, :])
            nc.sync.dma_start(nbs[0:C, :], nb_d[:, :])

            xtiles = []
            for i in range(NL):
                xt = xp.tile([128, LOAD_COLS], bf16, name=f"xt{i}", tag="xt")
                lo = LOAD_HALF[i] == "lo"
                psl = slice(0, 64) if lo else slice(64, 128)
                nc.sync.dma_start(xt[psl, :],
                                  xs_d[:, i * LOAD_COLS:(i + 1) * LOAD_COLS])
                xtiles.append((xt, lo))

            nc.vector.memset(dummy[:, :], 0.0)
            nc.scalar.activation(scratch[0:8, 0:8], dummy[0:8, 0:8], Exp,
                                 bias=0.0, scale=-1.0)

            warm = pp.tile([128, GROUP], f32, name="warm", tag="pt")
            for _ in range(10):
                nc.tensor.matmul(warm[0:C, 0:MM_F], lhsT=dummy[:, 0:C],
                                 rhs=dummy[:, :], start=True, stop=True)

            def emit_sq_mm(g):
                ti = (g * GROUP) // LOAD_COLS
                xt, lo = xtiles[ti]
                base = g * GROUP - ti * LOAD_COLS
                hsl = slice(base, base + GROUP)
                if lo:
                    nc.vector.tensor_mul(xt[64:128, hsl], xt[0:64, hsl],
                                         xt[0:64, hsl])
                    c0 = C
                else:
                    nc.vector.tensor_mul(xt[0:64, hsl], xt[64:128, hsl],
                                         xt[64:128, hsl])
                    c0 = 0
                pt = pp.tile([128, GROUP], f32, name=f"pt{g}", tag="pt")
                for q in range(GROUP // MM_F):
                    nc.tensor.matmul(
                        pt[0:C, q * MM_F:(q + 1) * MM_F],
                        lhsT=ws[:, c0:c0 + C],
                        rhs=xt[:, base + q * MM_F:base + (q + 1) * MM_F],
                        start=True, stop=True,
                    )
                return pt

            pts = {0: emit_sq_mm(0)}
            for g in range(NG):
                if g + 1 < NG:
                    pts[g + 1] = emit_sq_mm(g + 1)
                pt = pts.pop(g)
                ot = op.tile([128, GROUP], bf16, name=f"ot{g}", tag="ot")
                nc.scalar.activation(ot[0:C, :], pt[0:C, :], Exp,
                                     bias=nbs[0:C, :], scale=-1.0)
                nc.vector.tensor_scalar_max(ot[0:C, :], ot[0:C, :], 1e-6)
                osl = slice(g * GROUP, (g + 1) * GROUP)
                if g < NG - 1:
                    nc.gpsimd.dma_start(out_d[:, osl], ot[0:C, :])
                else:
                    nc.sync.dma_start(out_d[:, osl], ot[0:C, :])

    nc.compile()
    return nc


def get_nc():
    if "nc" not in _cache:
        _cache["nc"] = _build()
    return _cache["nc"]


def prep_in_maps(x, c, lamda):
    import ml_dtypes

    x = np.asarray(x, dtype=np.float32)
    c = np.asarray(c, dtype=np.float32)
    lamda = np.asarray(lamda, dtype=np.float32)

    lc = -2.0 * lamda * c
    w1 = np.concatenate([lamda, lc], axis=0)
    w2 = np.concatenate([lc, lamda], axis=0)
    w = np.concatenate([w1, w2], axis=1).astype(ml_dtypes.bfloat16)
    nb = (-np.sum(lamda * c * c, axis=0, dtype=np.float32)
          .astype(np.float32).reshape(C, 1))
    xb = x.astype(ml_dtypes.bfloat16)
    return [
        {"xs": np.ascontiguousarray(xb[n]), "w": w, "nb": nb}
        for n in range(N)
    ]


def kernel(x: np.ndarray, c: np.ndarray, lamda: np.ndarray) -> np.ndarray:
    from concourse.bass_utils import run_bass_kernel_spmd

    nc = get_nc()
    in_maps = prep_in_maps(x, c, lamda)
    res = run_bass_kernel_spmd(nc, in_maps, list(range(N)))
    out = np.stack([res.results[n]["out"] for n in range(N)], axis=0)
    return out.astype(np.float32)


if __name__ == "__main__":
    rng = np.random.default_rng(0)
    x = rng.standard_normal((N, D, WH), dtype=np.float32)
    c = rng.standard_normal((D, C), dtype=np.float32)
    lam = rng.random((D, C), dtype=np.float32)
    out = kernel(x, c, lam)
    print("out", out.shape, out.dtype, out.min(), out.max())

